# revision 2
# baseline (speedup 1.0000x reference)
"""Greedy online bipartite matching decode on 8 TRN2 NeuronCores (Bass/Tile).

Self-contained: hardcodes B=4096, V=256, U=128, skip=25, 8 cores, batch-sharded
(512 rows/core = 4 segments of 128 partitions).

Device per step t>=skip (batch rows on partitions):
  avail = w_t + pen ; m = max(0, max_j avail) ;
  code = (avail==m)*iota -> sel = sum(code) ; pen = min(code*-1e28, pen).
Exact f32 ties at the running max (possible with low-entropy RNG inputs)
corrupt sel/pen for that batch row; the host detects them via the invariant
w[t, sel-1] == m and recomputes affected rows exactly with numpy.
"""

import sys
from contextlib import ExitStack

import numpy as np

if "/opt/trn_rl_repo" not in sys.path:
    sys.path.insert(0, "/opt/trn_rl_repo")

import concourse.bacc as bacc
import concourse.mybir as mybir
import concourse.tile as tile
from concourse.bass_utils import run_bass_kernel_spmd

F32 = mybir.dt.float32
OP = mybir.AluOpType
AX = mybir.AxisListType

B = 4096
U = 128
V = 256
SKIP = 25
V_EFF = V - SKIP
NSEG = 4
BC = NSEG * 128
NCORES = 8
NEGC = -1e28
DMA_T = 4


def _build():
    nc = bacc.Bacc("TRN2", target_bir_lowering=False, debug=False)
    xr = nc.dram_tensor("xr", [V_EFF, BC, U], F32, kind="ExternalInput").ap()
    pi_enc = nc.dram_tensor("pi_enc", [128, NSEG * V], F32, kind="ExternalOutput").ap()
    m_out = nc.dram_tensor("m_out", [128, V_EFF * NSEG], F32, kind="ExternalOutput").ap()
    size_out = nc.dram_tensor("size_out", [128, NSEG], F32, kind="ExternalOutput").ap()

    with tile.TileContext(nc) as tc, ExitStack() as ctx:
        const_pool = ctx.enter_context(tc.tile_pool(name="const", bufs=1))
        state_pool = ctx.enter_context(tc.tile_pool(name="state", bufs=1))
        w_pool = ctx.enter_context(tc.tile_pool(name="w", bufs=3))
        a_pool = ctx.enter_context(tc.tile_pool(name="avail", bufs=4))

        iota_t = const_pool.tile([128, U], F32)
        nc.gpsimd.iota(
            iota_t[:], pattern=[[1, U]], base=1, channel_multiplier=0,
            allow_small_or_imprecise_dtypes=True,
        )
        pen = state_pool.tile([128, NSEG * U], F32)
        nc.vector.memset(pen[:], 0.0)
        m_buf = state_pool.tile([128, V_EFF * NSEG], F32)
        pi_buf = state_pool.tile([128, NSEG * V], F32)
        nc.vector.memset(pi_buf[:], 0.0)

        for tb in range(0, V_EFF, DMA_T):
            T = min(DMA_T, V_EFF - tb)
            wt = w_pool.tile([128, DMA_T * NSEG * U], F32, tag="wt")
            src = xr[tb : tb + T].rearrange("t (s p) c -> p t s c", p=128)
            dst = wt[:, : T * NSEG * U].rearrange(
                "p (t s c) -> p t s c", t=T, s=NSEG, c=U
            )
            nc.sync.dma_start(out=dst, in_=src)

            for k in range(T):
                t = tb + k
                avail = a_pool.tile([128, NSEG * U], F32, tag="avail")
                mcols = m_buf[:, t * NSEG : (t + 1) * NSEG]
                nc.vector.tensor_add(
                    avail[:], wt[:, k * NSEG * U : (k + 1) * NSEG * U], pen[:]
                )
                nc.vector.tensor_reduce(
                    out=mcols,
                    in_=avail[:].rearrange("p (s c) -> p s c", s=NSEG),
                    axis=AX.X,
                    op=OP.max,
                )
                nc.vector.tensor_scalar_max(mcols, mcols, 0.0)
                for s in range(NSEG):
                    ss = slice(s * U, (s + 1) * U)
                    nc.vector.scalar_tensor_tensor(
                        out=avail[:, ss],
                        in0=avail[:, ss],
                        scalar=m_buf[:, t * NSEG + s : t * NSEG + s + 1],
                        in1=iota_t[:],
                        op0=OP.is_equal,
                        op1=OP.mult,
                        accum_out=pi_buf[:, s * V + SKIP + t : s * V + SKIP + t + 1],
                    )
                nc.vector.scalar_tensor_tensor(
                    out=pen[:], in0=avail[:], scalar=NEGC, in1=pen[:],
                    op0=OP.mult, op1=OP.min,
                )

        size_t = state_pool.tile([128, NSEG], F32)
        nc.vector.tensor_reduce(
            out=size_t[:],
            in_=m_buf[:].rearrange("p (t s) -> p s t", s=NSEG),
            axis=AX.X,
            op=OP.add,
        )
        nc.sync.dma_start(out=pi_enc, in_=pi_buf[:])
        nc.sync.dma_start(out=m_out, in_=m_buf[:])
        nc.sync.dma_start(out=size_out, in_=size_t[:])
    nc.compile()
    return nc


_NC = None


def _get_nc():
    global _NC
    if _NC is None:
        _NC = _build()
    return _NC


def _host_row_fix(w_rows):
    """Exact greedy (reference semantics) for rows [n, V_EFF, U] -> (pi_eff, size)."""
    n = w_rows.shape[0]
    pen = np.zeros((n, U), np.float32)
    size = np.zeros((n,), np.float32)
    pi = np.zeros((n, V_EFF), np.int32)
    for t in range(V_EFF):
        avail = np.where(pen < 0, np.float32(-1e30), w_rows[:, t, :])
        am = np.argmax(avail, axis=1)
        mv = avail[np.arange(n), am]
        take = mv > 0
        sel = np.where(take, am + 1, 0)
        pi[:, t] = sel
        chosen = np.where(take, w_rows[:, t, :][np.arange(n), am], np.float32(0.0))
        size = (size + chosen.astype(np.float32)).astype(np.float32)
        pen[take, am[take]] = -1.0
    return pi, size


def kernel(x, u_size, v_size, skip_steps):
    assert x.shape == (B, V, U + 1) and int(u_size) == U
    assert int(v_size) == V and int(skip_steps) == SKIP
    x = np.ascontiguousarray(np.asarray(x, dtype=np.float32))
    nc = _get_nc()

    in_maps = []
    w_cores = []
    for c in range(NCORES):
        xc = x[c * BC : (c + 1) * BC, SKIP:, 1:]          # [BC, V_EFF, U]
        w_cores.append(xc)
        in_maps.append({"xr": np.ascontiguousarray(xc.transpose(1, 0, 2))})

    res = run_bass_kernel_spmd(nc, in_maps, core_ids=list(range(NCORES)))

    pi_full = np.zeros((B, V), np.int32)
    neg_size_full = np.zeros((B,), np.float32)
    for c in range(NCORES):
        r = res.results[c]
        pi_f = (
            r["pi_enc"].reshape(128, NSEG, V).transpose(1, 0, 2).reshape(BC, V)
        )
        m_f = (
            r["m_out"].reshape(128, V_EFF, NSEG).transpose(2, 0, 1).reshape(BC, V_EFF)
        )
        size_f = r["size_out"].T.reshape(BC)

        # Tie detection: for every effective step, sel>0 must point at a column
        # whose raw weight equals the recorded max; sel==0 requires m==0.
        w = w_cores[c]  # [BC, V_EFF, U]
        sel = pi_f[:, SKIP:].astype(np.int64)
        valid_idx = (sel >= 1) & (sel <= U)
        gather = np.take_along_axis(
            w, np.clip(sel - 1, 0, U - 1)[:, :, None], axis=2
        )[:, :, 0]
        ok = np.where(
            sel == 0,
            m_f == 0.0,
            valid_idx & (gather == m_f),
        )
        bad = ~ok.all(axis=1)
        if bad.any():
            pi_fix, size_fix = _host_row_fix(w[bad])
            pi_f[bad, SKIP:] = pi_fix
            pi_f[bad, :SKIP] = 0
            size_f = size_f.copy()
            size_f[bad] = size_fix

        pi_full[c * BC : (c + 1) * BC] = pi_f
        neg_size_full[c * BC : (c + 1) * BC] = -size_f

    return neg_size_full, pi_full
